# revision 44
# baseline (speedup 1.0000x reference)
"""CoordAttention kernel for Trainium2 (8 NeuronCores, data-parallel over batch).

Reference computation (per sample, inference):
  pools:  mean/max over W -> [C,H];  mean/max over H -> [C,W];  global mean/max -> [C]
  x_cat = concat(pools) -> [C, H+W+1, 2]
  y = BN(conv1x1(x_cat)) ; y = h_swish(y)
  a_h = sigmoid(conv(xh)), a_w = sigmoid(conv(xw)), a_c = sigmoid(conv(xc))
  out = x * a_w * a_h * a_c

v2 mapping onto one NeuronCore (2 samples each), fp16 datapath:
  - x loaded via casting SWDGE DMA (HBM fp32 -> SBUF fp16), chunked
    [128ch x 64h x 128w]
  - sum-pools: fp16 identity matmuls on PE (512-free, PSUM fp32 accum)
  - max-pools: fp16 tensor_tensor halving trees on DVE (2x fast mode)
  - conv1/BN folded host-side (fp32, tiny); sigmoids emit fp16
  - final multiply: ACT replicates a_h*a_c over w, DVE does two packed
    fp16 multiplies (2x); casting SWDGE DMA stores fp16 -> fp32 HBM
"""
import sys

for _p in ("/opt/trn_rl_repo", "/root/.axon_site/_ro/trn_rl_repo"):
    if _p not in sys.path:
        sys.path.insert(0, _p)

import copy as _copy
import numpy as np

import concourse.bass as bass
import concourse.mybir as mybir
import concourse.tile as tile

f32 = mybir.dt.float32
f16 = mybir.dt.float16
OP = mybir.AluOpType
AF = mybir.ActivationFunctionType
AX = mybir.AxisListType

N, C, H, W = 16, 256, 128, 128
TC = 8
NCORES = 8
PER = N // NCORES
EPS = 1e-5
CT = C // 128          # channel tiles per sample
NCH = 2                # h-chunks per channel tile
R = H // NCH           # rows per chunk
SS = H + W + 1         # pooled sequence length
JJ = 4                 # h-groups per sum-over-h matmul (free = JJ*W = 512)
MB = 8                 # w columns left after the PE sum-w stage


def _split_excess_waits(nc, limit=1):
    """This container's walrus accepts only one sync-wait per instruction;
    hoist extras onto same-engine drain carriers inserted just before."""
    m = nc.m
    newm = _copy.replace(m, functions=[])
    for fn in m.functions:
        newfn = _copy.replace(fn, blocks=[])
        newfn.set_allocations_from_list(fn.allocations)
        for blk in fn.blocks:
            out = []
            for inst in blk.instructions:
                si = inst.sync_info
                waits = list(si.on_wait) if si and si.on_wait else []
                if len(waits) > limit:
                    keep, excess = waits[-limit:], waits[: len(waits) - limit]
                    for gi, wchunk in enumerate(excess):
                        d = mybir.InstDrain(
                            name=f"{inst.name}-wsplit{gi}", ins=[], outs=[]
                        )
                        d.engine = inst.engine
                        d.sync_info = mybir.SyncInfo(on_wait=[wchunk], on_update=[])
                        out.append(d)
                    inst.sync_info = mybir.SyncInfo(
                        on_wait=keep, on_update=list(si.on_update or [])
                    )
                out.append(inst)
            newfn.blocks.append(_copy.replace(blk, instructions=out))
        newm.functions.append(newfn)
    nc.m = newm


def build_nc(per=PER, xp_bufs=8, split_waits=True):
    nc = bass.Bass()
    x_d = nc.declare_dram_parameter("x", [per, C, H, W], f32, isOutput=False)
    o_d = nc.declare_dram_parameter("out", [per, C, H, W], f32, isOutput=True)
    id_d = nc.declare_dram_parameter("ident16", [128, 128], f16, isOutput=False)
    w1_d = nc.declare_dram_parameter("w1t", [C, TC], f32, isOutput=False)
    b1_d = nc.declare_dram_parameter("b1f", [TC, 1], f32, isOutput=False)
    w2_d = nc.declare_dram_parameter("w2t", [TC, 2, C], f32, isOutput=False)
    w3_d = nc.declare_dram_parameter("w3t", [TC, 2, C], f32, isOutput=False)
    w4_d = nc.declare_dram_parameter("w4t", [TC, 2, C], f32, isOutput=False)
    b2_d = nc.declare_dram_parameter("b2r", [C, 1], f32, isOutput=False)
    b3_d = nc.declare_dram_parameter("b3r", [C, 1], f32, isOutput=False)
    b4_d = nc.declare_dram_parameter("b4r", [C, 1], f32, isOutput=False)
    cs_d = nc.declare_dram_parameter("colsc", [TC, SS], f32, isOutput=False)

    with tile.TileContext(nc) as tc:
        with (
            tc.tile_pool(name="const", bufs=1) as cp,
            tc.tile_pool(name="xp", bufs=xp_bufs) as xp,
            tc.tile_pool(name="scrp", bufs=3) as scr,
            tc.tile_pool(name="xcp", bufs=2) as xcp,
            tc.tile_pool(name="smp", bufs=2) as smp,
            tc.tile_pool(name="atp", bufs=3) as atp,
            tc.tile_pool(name="arp", bufs=2) as arp,
            tc.tile_pool(name="spool", bufs=2, space="PSUM") as spool,
            tc.tile_pool(name="pwpool", bufs=3, space="PSUM") as pwpool,
            tc.tile_pool(name="apsum", bufs=1, space="PSUM") as apsum,
        ):
            ident = cp.tile([128, 128], f16)
            nc.sync.dma_start(out=ident, in_=id_d[:, :])
            w1sb = cp.tile([128, CT, TC], f32)
            nc.sync.dma_start(
                out=w1sb, in_=w1_d.rearrange("(ct c) t -> c ct t", ct=CT)
            )
            b1sb = cp.tile([TC, 1], f32)
            nc.sync.dma_start(out=b1sb, in_=b1_d[:, :])
            colsc = cp.tile([TC, SS], f32)
            nc.sync.dma_start(out=colsc, in_=cs_d[:, :])
            wsb = {}
            for nm, d in (("w2", w2_d), ("w3", w3_d), ("w4", w4_d)):
                t = cp.tile([TC, 2, C], f32, tag=f"wsb_{nm}")
                nc.sync.dma_start(out=t, in_=d[:, :, :])
                wsb[nm] = t
            bsb = {}
            for nm, d in (("b2", b2_d), ("b3", b3_d), ("b4", b4_d)):
                t = cp.tile([128, CT, 1], f32, tag=f"bsb_{nm}")
                nc.sync.dma_start(out=t, in_=d.rearrange("(ct c) one -> c ct one", ct=CT))
                bsb[nm] = t

            # PE observes the identity once, so later matmuls carry one wait.
            warm = apsum.tile([128, 128], f32, tag="ap")
            nc.tensor.matmul(warm, ident, ident, start=True, stop=True)

            def load_chunk(s, ct, hh):
                # two half-loads per chunk: halves the first-tile latency
                # (the DMA engine pool runs ~2 transfers concurrently)
                t = xp.tile([128, R, W], f16, tag="x")
                for q in range(2):
                    h0 = hh * R + q * (R // 2)
                    nc.gpsimd.dma_start(
                        out=t[:, q * (R // 2) : (q + 1) * (R // 2), :],
                        in_=x_d[s, ct * 128 : (ct + 1) * 128, h0 : h0 + R // 2, :],
                    )
                return t

            xt = {}
            xc0, xc1 = {}, {}
            att = {}

            pstate = {}

            def emit_pools_pe(s, ct):
                """PE-only stream: sum-h accumulate + sum-w partial sums."""
                ch0, ch1 = xt[s, ct, 0], xt[s, ct, 1]
                xcat0 = xcp.tile([128, SS], f32, tag="xc0")
                xcat1 = xcp.tile([128, SS], f32, tag="xc1")
                psh = spool.tile([128, JJ, W], f32, tag="psh")
                # rhs reads JJ consecutive h-rows (1KB contiguous runs)
                for hh, ch in ((0, ch0), (1, ch1)):
                    chv = ch.rearrange("p (g jj) w -> p g jj w", jj=JJ)
                    gmax = R // JJ
                    for g in range(gmax):
                        nc.tensor.matmul(
                            psh,
                            ident,
                            chv[:, g],
                            start=(hh == 0 and g == 0),
                            stop=(hh == 1 and g == gmax - 1),
                        )
                pws = []
                for hh, ch in ((0, ch0), (1, ch1)):
                    pw = pwpool.tile([128, R, MB], f32, tag="pw")
                    for j in range(W // MB):
                        nc.tensor.matmul(
                            pw,
                            ident,
                            ch[:, :, j * MB : (j + 1) * MB],
                            start=(j == 0),
                            stop=(j == W // MB - 1),
                        )
                    pws.append(pw)
                pstate[s, ct] = (xcat0, xcat1, psh, pws)

            def emit_pools_trees(s, ct):
                """max-w trees for both chunks (no PE dependency)."""
                ch0, ch1 = xt[s, ct, 0], xt[s, ct, 1]
                xcat0, xcat1, psh, pws = pstate[s, ct]
                for hh, ch in ((0, ch0), (1, ch1)):
                    t = scr.tile([128, R * W // 2], f16, tag="scr")
                    t = t.rearrange("p (h w) -> p h w", h=R)
                    nc.vector.tensor_tensor(
                        out=t, in0=ch[:, :, 0 : W // 2], in1=ch[:, :, W // 2 : W],
                        op=OP.max,
                    )
                    g = W // 2
                    while g > 8:
                        nc.vector.tensor_tensor(
                            out=t[:, :, 0 : g // 2],
                            in0=t[:, :, 0 : g // 2],
                            in1=t[:, :, g // 2 : g],
                            op=OP.max,
                        )
                        g //= 2
                    nc.vector.reduce_max(
                        out=xcat1[:, hh * R : (hh + 1) * R], in_=t[:, :, 0:8],
                        axis=AX.X,
                    )

            def emit_pools_reduces(s, ct):
                """sum-w reduces straight from PSUM (needs PE stage done)."""
                xcat0, xcat1, psh, pws = pstate[s, ct]
                swv = xcat0[:, 0:H].rearrange("p (hh r) -> p hh r", hh=NCH)
                for hh in range(NCH):
                    nc.vector.reduce_sum(out=swv[:, hh], in_=pws[hh], axis=AX.X)
                # raw global sum (reads the sums just written)
                nc.vector.reduce_sum(
                    out=xcat0[:, H + W : SS], in_=xcat0[:, 0:H], axis=AX.X
                )

            def emit_pools_dve_a(s, ct):
                emit_pools_trees(s, ct)
                emit_pools_reduces(s, ct)

            def emit_pools_dve_b(s, ct):
                ch0, ch1 = xt[s, ct, 0], xt[s, ct, 1]
                xcat0, xcat1, psh, pws = pstate[s, ct]
                # max over h (DVE f16 tree): cross-chunk then halve
                mh = scr.tile([128, R * W // 2], f16, tag="scr")
                mh = mh.rearrange("p (h w) -> p h w", h=R // 2)
                nc.vector.tensor_tensor(
                    out=mh, in0=ch0[:, 0 : R // 2, :], in1=ch0[:, R // 2 : R, :],
                    op=OP.max,
                )
                nc.vector.tensor_tensor(
                    out=mh, in0=mh,
                    in1=ch1.rearrange("p (a b) w -> p a b w", a=2)[:, 0],
                    op=OP.max,
                )
                nc.vector.tensor_tensor(
                    out=mh, in0=mh,
                    in1=ch1.rearrange("p (a b) w -> p a b w", a=2)[:, 1],
                    op=OP.max,
                )
                g = R // 2
                while g > 1:
                    nc.vector.tensor_tensor(
                        out=mh[:, 0 : g // 2, :],
                        in0=mh[:, 0 : g // 2, :],
                        in1=mh[:, g // 2 : g, :],
                        op=OP.max,
                    )
                    g //= 2
                nc.vector.tensor_copy(out=xcat1[:, H : H + W], in_=mh[:, 0, :])
                # global max
                nc.vector.reduce_max(
                    out=xcat1[:, H + W : SS], in_=xcat1[:, 0:H], axis=AX.X
                )
                # fold sum-h partials (PSUM -> xcat0, raw sums)
                sh = xcat0[:, H : H + W]
                nc.vector.tensor_copy(out=sh, in_=psh[:, 0, :])
                nc.vector.tensor_add(out=sh, in0=sh, in1=psh[:, 1, :])
                nc.vector.tensor_add(out=sh, in0=sh, in1=psh[:, 2, :])
                nc.vector.tensor_add(out=sh, in0=sh, in1=psh[:, 3, :])
                xc0[s, ct], xc1[s, ct] = xcat0, xcat1

            cstate = {}

            def emit_conv_pe(s):
                # conv1 matmuls only (PE); mean scaling folded per-column
                xh = smp.tile([TC, 2, SS], f32, tag="xh")
                yps = []
                for k in range(2):
                    yp = apsum.tile([TC, SS], f32, tag="y")
                    src = xc0 if k == 0 else xc1
                    for ct in range(CT):
                        nc.tensor.matmul(
                            yp,
                            w1sb[:, ct, :],
                            src[s, ct],
                            start=(ct == 0),
                            stop=(ct == CT - 1),
                        )
                    yps.append(yp)
                cstate[s] = (xh, yps)

            def emit_conv_dve(s):
                # bias + h_swish pieces (DVE)
                xh, yps = cstate[s]
                for k in range(2):
                    yp = yps[k]
                    xhk = xh[:, k, :]
                    if k == 0:
                        nc.vector.tensor_mul(out=xhk, in0=yp, in1=colsc)
                        nc.vector.tensor_scalar_add(out=xhk, in0=xhk, scalar1=b1sb)
                    else:
                        nc.vector.tensor_scalar_add(out=xhk, in0=yp, scalar1=b1sb)
                    u = smp.tile([TC, SS], f32, tag="u")
                    nc.vector.tensor_scalar(
                        out=u, in0=xhk, scalar1=-3.0, scalar2=3.0,
                        op0=OP.max, op1=OP.min,
                    )
                    nc.vector.tensor_scalar_add(out=u, in0=u, scalar1=3.0)
                    nc.vector.tensor_mul(out=xhk, in0=xhk, in1=u)

            def emit_att(s):
                # attention maps (sigmoids in f16)
                xh, _ = cstate[s]
                for ct in range(CT):
                    ah = atp.tile([128, H], f16, tag="ah")
                    aw = atp.tile([128, W], f16, tag="aw")
                    ac = atp.tile([128, 1], f32, tag="ac")
                    for nm, bt, sl, dst in (
                        ("w2", "b2", slice(0, H), ah),
                        ("w3", "b3", slice(H, H + W), aw),
                        ("w4", "b4", slice(H + W, SS), ac),
                    ):
                        pp = apsum.tile([128, H], f32, tag="ap")
                        po = pp[:, 0 : dst.shape[-1]]
                        for k in range(2):
                            nc.tensor.matmul(
                                po,
                                wsb[nm][:, k, ct * 128 : (ct + 1) * 128],
                                xh[:, k, sl],
                                start=(k == 0),
                                stop=(k == 1),
                            )
                        nc.scalar.activation(
                            out=dst, in_=po, func=AF.Sigmoid,
                            bias=bsb[bt][:, ct, :], scale=1.0,
                        )
                    # ahc = a_h * a_c (f16, per-partition scalar)
                    nc.vector.tensor_scalar_mul(out=ah, in0=ah, scalar1=ac)
                    att[s, ct] = (ah, aw)
                    if ct == 0:
                        emit_repl(s, 0)

            reps = {}

            def emit_repl(s, ct):
                ah, _ = att[s, ct]
                for hh in range(NCH):
                    ahcrep = arp.tile([128, R, W], f16, tag="ahcrep")
                    nc.scalar.activation(
                        out=ahcrep,
                        in_=ah[:, hh * R : (hh + 1) * R]
                        .unsqueeze(2)
                        .to_broadcast([128, R, W]),
                        func=AF.Copy,
                    )
                    reps[s, ct, hh] = ahcrep

            def emit_apply(s, ct, hh, bcast_first=False):
                ah, aw = att[s, ct]
                HF = R // 2
                ch = xt[s, ct, hh]
                for half in range(2):
                    sl = slice(half * HF, (half + 1) * HF)
                    # x *= a_w (bcast mid), then *= ahc (packed) -- both 2x
                    nc.vector.tensor_tensor(
                        out=ch[:, sl],
                        in0=ch[:, sl],
                        in1=aw.unsqueeze(1).to_broadcast([128, HF, W]),
                        op=OP.mult,
                    )
                    if bcast_first and half == 0:
                        # stride-0 broadcast (1x) -- skips the repl dependency
                        # right after the attention chain
                        hc = ah[:, hh * R + half * HF : hh * R + (half + 1) * HF]
                        in1 = hc.unsqueeze(2).to_broadcast([128, HF, W])
                    else:
                        in1 = reps[s, ct, hh][:, sl]
                    nc.vector.tensor_tensor(
                        out=ch[:, sl], in0=ch[:, sl], in1=in1, op=OP.mult,
                    )
                    # casting store: f16 SBUF -> fp32 HBM
                    nc.gpsimd.dma_start(
                        out=o_d[
                            s,
                            ct * 128 : (ct + 1) * 128,
                            hh * R + half * HF : hh * R + (half + 1) * HF,
                            :,
                        ],
                        in_=ch[:, sl],
                    )

            # ---- software-pipelined schedule (per = 2) ----
            for s in range(per):
                for ct in range(CT):
                    for hh in range(NCH):
                        xt[s, ct, hh] = load_chunk(s, ct, hh)
            emit_pools_pe(0, 0)
            emit_pools_dve_a(0, 0)
            emit_pools_dve_b(0, 0)
            emit_pools_pe(0, 1)
            emit_pools_trees(0, 1)
            emit_pools_reduces(0, 1)
            emit_pools_dve_b(0, 1)
            emit_pools_pe(1, 0)       # PE: before conv1 (conv1 waits on DVE)
            emit_pools_trees(1, 0)    # DVE: fills the conv(0)-chain latency
            emit_conv_pe(0)
            emit_conv_dve(0)
            emit_att(0)               # includes repl(0, 0)
            emit_pools_pe(1, 1)
            emit_apply(0, 0, 0, bcast_first=True)
            emit_apply(0, 0, 1)
            emit_repl(0, 1)
            emit_apply(0, 1, 0)
            emit_apply(0, 1, 1)
            emit_pools_reduces(1, 0)
            emit_pools_dve_b(1, 0)
            emit_pools_trees(1, 1)
            emit_pools_reduces(1, 1)
            emit_pools_dve_b(1, 1)
            emit_conv_pe(1)
            emit_conv_dve(1)
            emit_att(1)               # includes repl(1, 0)
            emit_apply(1, 0, 0, bcast_first=True)
            emit_apply(1, 0, 1)
            emit_repl(1, 1)
            emit_apply(1, 1, 0)
            emit_apply(1, 1, 1)

    if split_waits:
        _split_excess_waits(nc)
    return nc


def prep_weights(w1, b1, bn_gamma, bn_beta, bn_mean, bn_var, w2, b2, w3, b3, w4, b4):
    inv = (bn_gamma / np.sqrt(bn_var + EPS)).astype(np.float32)
    w1f = (w1 * inv[:, None]).astype(np.float32)          # [TC, C]
    b1f = ((b1 - bn_mean) * inv + bn_beta).astype(np.float32)
    def pack(wk):  # [C, TC, 2] -> [TC, 2, C], with the h_swish /6 folded in
        return np.ascontiguousarray(wk.transpose(1, 2, 0) / 6.0).astype(np.float32)
    colvec = np.concatenate([
        np.full(H, 1.0 / W), np.full(W, 1.0 / H), np.full(1, 1.0 / (H * W))
    ]).astype(np.float32)
    return dict(
        ident16=np.eye(128, dtype=np.float16),
        w1t=np.ascontiguousarray(w1f.T),                  # [C, TC]
        b1f=b1f.reshape(TC, 1),
        colsc=np.ascontiguousarray(np.tile(colvec, (TC, 1))),
        w2t=pack(w2), w3t=pack(w3), w4t=pack(w4),
        b2r=b2.reshape(C, 1).astype(np.float32),
        b3r=b3.reshape(C, 1).astype(np.float32),
        b4r=b4.reshape(C, 1).astype(np.float32),
    )


_NC_CACHE = {}


def _get_nc():
    if "nc" not in _NC_CACHE:
        _NC_CACHE["nc"] = build_nc()
    return _NC_CACHE["nc"]


def kernel(x, w1, b1, bn_gamma, bn_beta, bn_mean, bn_var, w2, b2, w3, b3, w4, b4):
    from concourse.bass_utils import run_bass_kernel_spmd

    x = np.asarray(x, dtype=np.float32)
    wmap = prep_weights(
        np.asarray(w1, np.float32), np.asarray(b1, np.float32),
        np.asarray(bn_gamma, np.float32), np.asarray(bn_beta, np.float32),
        np.asarray(bn_mean, np.float32), np.asarray(bn_var, np.float32),
        np.asarray(w2, np.float32), np.asarray(b2, np.float32),
        np.asarray(w3, np.float32), np.asarray(b3, np.float32),
        np.asarray(w4, np.float32), np.asarray(b4, np.float32),
    )
    nc = _get_nc()
    in_maps = [
        {"x": np.ascontiguousarray(x[i * PER : (i + 1) * PER]), **wmap}
        for i in range(NCORES)
    ]
    res = run_bass_kernel_spmd(nc, in_maps, core_ids=list(range(NCORES)))
    return np.concatenate([res.results[i]["out"] for i in range(NCORES)], axis=0)


# revision 45
# speedup vs baseline: 1.0367x; 1.0367x over previous
"""CoordAttention kernel for Trainium2 (8 NeuronCores, data-parallel over batch).

Reference computation (per sample, inference):
  pools:  mean/max over W -> [C,H];  mean/max over H -> [C,W];  global mean/max -> [C]
  x_cat = concat(pools) -> [C, H+W+1, 2]
  y = BN(conv1x1(x_cat)) ; y = h_swish(y)
  a_h = sigmoid(conv(xh)), a_w = sigmoid(conv(xw)), a_c = sigmoid(conv(xc))
  out = x * a_w * a_h * a_c

v2 mapping onto one NeuronCore (2 samples each), fp16 datapath:
  - x loaded via casting SWDGE DMA (HBM fp32 -> SBUF fp16), chunked
    [128ch x 64h x 128w]
  - sum-pools: fp16 identity matmuls on PE (512-free, PSUM fp32 accum)
  - max-pools: fp16 tensor_tensor halving trees on DVE (2x fast mode)
  - conv1/BN folded host-side (fp32, tiny); sigmoids emit fp16
  - final multiply: ACT replicates a_h*a_c over w, DVE does two packed
    fp16 multiplies (2x); casting SWDGE DMA stores fp16 -> fp32 HBM
"""
import sys

for _p in ("/opt/trn_rl_repo", "/root/.axon_site/_ro/trn_rl_repo"):
    if _p not in sys.path:
        sys.path.insert(0, _p)

import copy as _copy
import numpy as np

import concourse.bass as bass
import concourse.mybir as mybir
import concourse.tile as tile

f32 = mybir.dt.float32
f16 = mybir.dt.float16
OP = mybir.AluOpType
AF = mybir.ActivationFunctionType
AX = mybir.AxisListType

N, C, H, W = 16, 256, 128, 128
TC = 8
NCORES = 8
PER = N // NCORES
EPS = 1e-5
CT = C // 128          # channel tiles per sample
NCH = 2                # h-chunks per channel tile
R = H // NCH           # rows per chunk
SS = H + W + 1         # pooled sequence length
JJ = 4                 # h-groups per sum-over-h matmul (free = JJ*W = 512)
MB = 8                 # w columns left after the PE sum-w stage


def _split_excess_waits(nc, limit=1):
    """This container's walrus accepts only one sync-wait per instruction;
    hoist extras onto same-engine drain carriers inserted just before."""
    m = nc.m
    newm = _copy.replace(m, functions=[])
    for fn in m.functions:
        newfn = _copy.replace(fn, blocks=[])
        newfn.set_allocations_from_list(fn.allocations)
        for blk in fn.blocks:
            out = []
            for inst in blk.instructions:
                si = inst.sync_info
                waits = list(si.on_wait) if si and si.on_wait else []
                if len(waits) > limit:
                    keep, excess = waits[-limit:], waits[: len(waits) - limit]
                    for gi, wchunk in enumerate(excess):
                        d = mybir.InstDrain(
                            name=f"{inst.name}-wsplit{gi}", ins=[], outs=[]
                        )
                        d.engine = inst.engine
                        d.sync_info = mybir.SyncInfo(on_wait=[wchunk], on_update=[])
                        out.append(d)
                    inst.sync_info = mybir.SyncInfo(
                        on_wait=keep, on_update=list(si.on_update or [])
                    )
                out.append(inst)
            newfn.blocks.append(_copy.replace(blk, instructions=out))
        newm.functions.append(newfn)
    nc.m = newm


def build_nc(per=PER, xp_bufs=8, split_waits=True):
    nc = bass.Bass()
    x_d = nc.declare_dram_parameter("x", [per, C, H, W], f32, isOutput=False)
    o_d = nc.declare_dram_parameter("out", [per, C, H, W], f32, isOutput=True)
    id_d = nc.declare_dram_parameter("ident16", [128, 128], f16, isOutput=False)
    w1_d = nc.declare_dram_parameter("w1t", [C, TC], f32, isOutput=False)
    b1_d = nc.declare_dram_parameter("b1f", [TC, 1], f32, isOutput=False)
    w2_d = nc.declare_dram_parameter("w2t", [TC, 2, C], f32, isOutput=False)
    w3_d = nc.declare_dram_parameter("w3t", [TC, 2, C], f32, isOutput=False)
    w4_d = nc.declare_dram_parameter("w4t", [TC, 2, C], f32, isOutput=False)
    b2_d = nc.declare_dram_parameter("b2r", [C, 1], f32, isOutput=False)
    b3_d = nc.declare_dram_parameter("b3r", [C, 1], f32, isOutput=False)
    b4_d = nc.declare_dram_parameter("b4r", [C, 1], f32, isOutput=False)
    cs_d = nc.declare_dram_parameter("colsc", [TC, SS], f32, isOutput=False)

    with tile.TileContext(nc) as tc:
        with (
            tc.tile_pool(name="const", bufs=1) as cp,
            tc.tile_pool(name="xp", bufs=xp_bufs) as xp,
            tc.tile_pool(name="scrp", bufs=3) as scr,
            tc.tile_pool(name="xcp", bufs=2) as xcp,
            tc.tile_pool(name="smp", bufs=2) as smp,
            tc.tile_pool(name="atp", bufs=3) as atp,
            tc.tile_pool(name="arp", bufs=2) as arp,
            tc.tile_pool(name="spool", bufs=2, space="PSUM") as spool,
            tc.tile_pool(name="pwpool", bufs=2, space="PSUM") as pwpool,
            tc.tile_pool(name="apsum", bufs=2, space="PSUM") as apsum,
        ):
            ident = cp.tile([128, 128], f16)
            nc.sync.dma_start(out=ident, in_=id_d[:, :])
            w1sb = cp.tile([128, CT, TC], f32)
            nc.sync.dma_start(
                out=w1sb, in_=w1_d.rearrange("(ct c) t -> c ct t", ct=CT)
            )
            b1sb = cp.tile([TC, 1], f32)
            nc.sync.dma_start(out=b1sb, in_=b1_d[:, :])
            colsc = cp.tile([TC, SS], f32)
            nc.sync.dma_start(out=colsc, in_=cs_d[:, :])
            wsb = {}
            for nm, d in (("w2", w2_d), ("w3", w3_d), ("w4", w4_d)):
                t = cp.tile([TC, 2, C], f32, tag=f"wsb_{nm}")
                nc.sync.dma_start(out=t, in_=d[:, :, :])
                wsb[nm] = t
            bsb = {}
            for nm, d in (("b2", b2_d), ("b3", b3_d), ("b4", b4_d)):
                t = cp.tile([128, CT, 1], f32, tag=f"bsb_{nm}")
                nc.sync.dma_start(out=t, in_=d.rearrange("(ct c) one -> c ct one", ct=CT))
                bsb[nm] = t

            # PE observes the identity once, so later matmuls carry one wait.
            warm = apsum.tile([128, 128], f32, tag="ap")
            nc.tensor.matmul(warm, ident, ident, start=True, stop=True)

            def load_chunk(s, ct, hh):
                # two half-loads per chunk: halves the first-tile latency
                # (the DMA engine pool runs ~2 transfers concurrently)
                t = xp.tile([128, R, W], f16, tag="x")
                for q in range(2):
                    h0 = hh * R + q * (R // 2)
                    nc.gpsimd.dma_start(
                        out=t[:, q * (R // 2) : (q + 1) * (R // 2), :],
                        in_=x_d[s, ct * 128 : (ct + 1) * 128, h0 : h0 + R // 2, :],
                    )
                return t

            xt = {}
            xc0, xc1 = {}, {}
            att = {}

            pstate = {}

            def emit_pools_pe(s, ct):
                """PE-only stream: sum-h accumulate + sum-w partial sums."""
                ch0, ch1 = xt[s, ct, 0], xt[s, ct, 1]
                xcat0 = xcp.tile([128, SS], f32, tag="xc0")
                xcat1 = xcp.tile([128, SS], f32, tag="xc1")
                psh = spool.tile([128, JJ, W], f32, tag="psh")
                # rhs reads JJ consecutive h-rows (1KB contiguous runs)
                for hh, ch in ((0, ch0), (1, ch1)):
                    chv = ch.rearrange("p (g jj) w -> p g jj w", jj=JJ)
                    gmax = R // JJ
                    for g in range(gmax):
                        nc.tensor.matmul(
                            psh,
                            ident,
                            chv[:, g],
                            start=(hh == 0 and g == 0),
                            stop=(hh == 1 and g == gmax - 1),
                        )
                pws = []
                for hh, ch in ((0, ch0), (1, ch1)):
                    pw = pwpool.tile([128, R, MB], f32, tag="pw")
                    for j in range(W // MB):
                        nc.tensor.matmul(
                            pw,
                            ident,
                            ch[:, :, j * MB : (j + 1) * MB],
                            start=(j == 0),
                            stop=(j == W // MB - 1),
                        )
                    pws.append(pw)
                pstate[s, ct] = (xcat0, xcat1, psh, pws)

            def emit_pools_trees(s, ct):
                """max-w trees for both chunks (no PE dependency)."""
                ch0, ch1 = xt[s, ct, 0], xt[s, ct, 1]
                xcat0, xcat1, psh, pws = pstate[s, ct]
                for hh, ch in ((0, ch0), (1, ch1)):
                    t = scr.tile([128, R * W // 2], f16, tag="scr")
                    t = t.rearrange("p (h w) -> p h w", h=R)
                    nc.vector.tensor_tensor(
                        out=t, in0=ch[:, :, 0 : W // 2], in1=ch[:, :, W // 2 : W],
                        op=OP.max,
                    )
                    g = W // 2
                    while g > 8:
                        nc.vector.tensor_tensor(
                            out=t[:, :, 0 : g // 2],
                            in0=t[:, :, 0 : g // 2],
                            in1=t[:, :, g // 2 : g],
                            op=OP.max,
                        )
                        g //= 2
                    nc.vector.reduce_max(
                        out=xcat1[:, hh * R : (hh + 1) * R], in_=t[:, :, 0:8],
                        axis=AX.X,
                    )

            def emit_pools_reduces(s, ct):
                """sum-w reduces straight from PSUM (needs PE stage done)."""
                xcat0, xcat1, psh, pws = pstate[s, ct]
                swv = xcat0[:, 0:H].rearrange("p (hh r) -> p hh r", hh=NCH)
                for hh in range(NCH):
                    nc.vector.reduce_sum(out=swv[:, hh], in_=pws[hh], axis=AX.X)
                # raw global sum (reads the sums just written)
                nc.vector.reduce_sum(
                    out=xcat0[:, H + W : SS], in_=xcat0[:, 0:H], axis=AX.X
                )

            def emit_pools_dve_a(s, ct):
                emit_pools_trees(s, ct)
                emit_pools_reduces(s, ct)

            def emit_pools_dve_b(s, ct):
                ch0, ch1 = xt[s, ct, 0], xt[s, ct, 1]
                xcat0, xcat1, psh, pws = pstate[s, ct]
                # max over h (DVE f16 tree): cross-chunk then halve
                mh = scr.tile([128, R * W // 2], f16, tag="scr")
                mh = mh.rearrange("p (h w) -> p h w", h=R // 2)
                nc.vector.tensor_tensor(
                    out=mh, in0=ch0[:, 0 : R // 2, :], in1=ch0[:, R // 2 : R, :],
                    op=OP.max,
                )
                nc.vector.tensor_tensor(
                    out=mh, in0=mh,
                    in1=ch1.rearrange("p (a b) w -> p a b w", a=2)[:, 0],
                    op=OP.max,
                )
                nc.vector.tensor_tensor(
                    out=mh, in0=mh,
                    in1=ch1.rearrange("p (a b) w -> p a b w", a=2)[:, 1],
                    op=OP.max,
                )
                g = R // 2
                while g > 1:
                    nc.vector.tensor_tensor(
                        out=mh[:, 0 : g // 2, :],
                        in0=mh[:, 0 : g // 2, :],
                        in1=mh[:, g // 2 : g, :],
                        op=OP.max,
                    )
                    g //= 2
                nc.vector.tensor_copy(out=xcat1[:, H : H + W], in_=mh[:, 0, :])
                # global max
                nc.vector.reduce_max(
                    out=xcat1[:, H + W : SS], in_=xcat1[:, 0:H], axis=AX.X
                )
                # fold sum-h partials (PSUM -> xcat0, raw sums)
                sh = xcat0[:, H : H + W]
                nc.vector.tensor_copy(out=sh, in_=psh[:, 0, :])
                nc.vector.tensor_add(out=sh, in0=sh, in1=psh[:, 1, :])
                nc.vector.tensor_add(out=sh, in0=sh, in1=psh[:, 2, :])
                nc.vector.tensor_add(out=sh, in0=sh, in1=psh[:, 3, :])
                xc0[s, ct], xc1[s, ct] = xcat0, xcat1

            cstate = {}

            def emit_conv_pe(s):
                # conv1 matmuls only (PE); mean scaling folded per-column
                xh = smp.tile([TC, 2, SS], f32, tag="xh")
                yps = []
                for k in range(2):
                    yp = apsum.tile([TC, SS], f32, tag="y")
                    src = xc0 if k == 0 else xc1
                    for ct in range(CT):
                        nc.tensor.matmul(
                            yp,
                            w1sb[:, ct, :],
                            src[s, ct],
                            start=(ct == 0),
                            stop=(ct == CT - 1),
                        )
                    yps.append(yp)
                cstate[s] = (xh, yps)

            def emit_conv_dve(s):
                # bias + h_swish pieces (DVE)
                xh, yps = cstate[s]
                for k in range(2):
                    yp = yps[k]
                    xhk = xh[:, k, :]
                    if k == 0:
                        nc.vector.tensor_mul(out=xhk, in0=yp, in1=colsc)
                        nc.vector.tensor_scalar_add(out=xhk, in0=xhk, scalar1=b1sb)
                    else:
                        nc.vector.tensor_scalar_add(out=xhk, in0=yp, scalar1=b1sb)
                    u = smp.tile([TC, SS], f32, tag="u")
                    nc.vector.tensor_scalar(
                        out=u, in0=xhk, scalar1=-3.0, scalar2=3.0,
                        op0=OP.max, op1=OP.min,
                    )
                    nc.vector.tensor_scalar_add(out=u, in0=u, scalar1=3.0)
                    nc.vector.tensor_mul(out=xhk, in0=xhk, in1=u)

            def emit_att(s):
                # attention maps (sigmoids in f16)
                xh, _ = cstate[s]
                for ct in range(CT):
                    ah = atp.tile([128, H], f16, tag="ah")
                    aw = atp.tile([128, W], f16, tag="aw")
                    ac = atp.tile([128, 1], f32, tag="ac")
                    for nm, bt, sl, dst in (
                        ("w2", "b2", slice(0, H), ah),
                        ("w3", "b3", slice(H, H + W), aw),
                        ("w4", "b4", slice(H + W, SS), ac),
                    ):
                        pp = apsum.tile([128, H], f32, tag="ap")
                        po = pp[:, 0 : dst.shape[-1]]
                        for k in range(2):
                            nc.tensor.matmul(
                                po,
                                wsb[nm][:, k, ct * 128 : (ct + 1) * 128],
                                xh[:, k, sl],
                                start=(k == 0),
                                stop=(k == 1),
                            )
                        nc.scalar.activation(
                            out=dst, in_=po, func=AF.Sigmoid,
                            bias=bsb[bt][:, ct, :], scale=1.0,
                        )
                    # ahc = a_h * a_c (f16, per-partition scalar)
                    nc.vector.tensor_scalar_mul(out=ah, in0=ah, scalar1=ac)
                    att[s, ct] = (ah, aw)
                    if ct == 0:
                        emit_repl(s, 0)

            reps = {}

            def emit_repl(s, ct):
                ah, _ = att[s, ct]
                for hh in range(NCH):
                    ahcrep = arp.tile([128, R, W], f16, tag="ahcrep")
                    nc.scalar.activation(
                        out=ahcrep,
                        in_=ah[:, hh * R : (hh + 1) * R]
                        .unsqueeze(2)
                        .to_broadcast([128, R, W]),
                        func=AF.Copy,
                    )
                    reps[s, ct, hh] = ahcrep

            def emit_apply(s, ct, hh, bcast_first=False):
                ah, aw = att[s, ct]
                HF = R // 2
                ch = xt[s, ct, hh]
                for half in range(2):
                    sl = slice(half * HF, (half + 1) * HF)
                    # x *= a_w (bcast mid), then *= ahc (packed) -- both 2x
                    nc.vector.tensor_tensor(
                        out=ch[:, sl],
                        in0=ch[:, sl],
                        in1=aw.unsqueeze(1).to_broadcast([128, HF, W]),
                        op=OP.mult,
                    )
                    if bcast_first and half == 0:
                        # stride-0 broadcast (1x) -- skips the repl dependency
                        # right after the attention chain
                        hc = ah[:, hh * R + half * HF : hh * R + (half + 1) * HF]
                        in1 = hc.unsqueeze(2).to_broadcast([128, HF, W])
                    else:
                        in1 = reps[s, ct, hh][:, sl]
                    nc.vector.tensor_tensor(
                        out=ch[:, sl], in0=ch[:, sl], in1=in1, op=OP.mult,
                    )
                    # casting store: f16 SBUF -> fp32 HBM
                    nc.gpsimd.dma_start(
                        out=o_d[
                            s,
                            ct * 128 : (ct + 1) * 128,
                            hh * R + half * HF : hh * R + (half + 1) * HF,
                            :,
                        ],
                        in_=ch[:, sl],
                    )

            # ---- software-pipelined schedule (per = 2) ----
            for s in range(per):
                for ct in range(CT):
                    for hh in range(NCH):
                        xt[s, ct, hh] = load_chunk(s, ct, hh)
            emit_pools_pe(0, 0)
            emit_pools_dve_a(0, 0)
            emit_pools_dve_b(0, 0)
            emit_pools_pe(0, 1)
            emit_pools_trees(0, 1)
            emit_pools_reduces(0, 1)
            emit_pools_dve_b(0, 1)
            emit_pools_pe(1, 0)       # PE: before conv1 (conv1 waits on DVE)
            emit_pools_trees(1, 0)    # DVE: fills the conv(0)-chain latency
            emit_conv_pe(0)
            emit_conv_dve(0)
            emit_att(0)               # includes repl(0, 0)
            emit_pools_pe(1, 1)
            emit_apply(0, 0, 0, bcast_first=True)
            emit_apply(0, 0, 1)
            emit_repl(0, 1)
            emit_apply(0, 1, 0)
            emit_apply(0, 1, 1)
            emit_pools_reduces(1, 0)
            emit_pools_dve_b(1, 0)
            emit_pools_trees(1, 1)
            emit_pools_reduces(1, 1)
            emit_pools_dve_b(1, 1)
            emit_conv_pe(1)
            emit_conv_dve(1)
            emit_att(1)               # includes repl(1, 0)
            emit_apply(1, 0, 0, bcast_first=True)
            emit_apply(1, 0, 1)
            emit_repl(1, 1)
            emit_apply(1, 1, 0)
            emit_apply(1, 1, 1)

    if split_waits:
        _split_excess_waits(nc)
    return nc


def prep_weights(w1, b1, bn_gamma, bn_beta, bn_mean, bn_var, w2, b2, w3, b3, w4, b4):
    inv = (bn_gamma / np.sqrt(bn_var + EPS)).astype(np.float32)
    w1f = (w1 * inv[:, None]).astype(np.float32)          # [TC, C]
    b1f = ((b1 - bn_mean) * inv + bn_beta).astype(np.float32)
    def pack(wk):  # [C, TC, 2] -> [TC, 2, C], with the h_swish /6 folded in
        return np.ascontiguousarray(wk.transpose(1, 2, 0) / 6.0).astype(np.float32)
    colvec = np.concatenate([
        np.full(H, 1.0 / W), np.full(W, 1.0 / H), np.full(1, 1.0 / (H * W))
    ]).astype(np.float32)
    return dict(
        ident16=np.eye(128, dtype=np.float16),
        w1t=np.ascontiguousarray(w1f.T),                  # [C, TC]
        b1f=b1f.reshape(TC, 1),
        colsc=np.ascontiguousarray(np.tile(colvec, (TC, 1))),
        w2t=pack(w2), w3t=pack(w3), w4t=pack(w4),
        b2r=b2.reshape(C, 1).astype(np.float32),
        b3r=b3.reshape(C, 1).astype(np.float32),
        b4r=b4.reshape(C, 1).astype(np.float32),
    )


_NC_CACHE = {}


def _get_nc():
    if "nc" not in _NC_CACHE:
        _NC_CACHE["nc"] = build_nc()
    return _NC_CACHE["nc"]


def kernel(x, w1, b1, bn_gamma, bn_beta, bn_mean, bn_var, w2, b2, w3, b3, w4, b4):
    from concourse.bass_utils import run_bass_kernel_spmd

    x = np.asarray(x, dtype=np.float32)
    wmap = prep_weights(
        np.asarray(w1, np.float32), np.asarray(b1, np.float32),
        np.asarray(bn_gamma, np.float32), np.asarray(bn_beta, np.float32),
        np.asarray(bn_mean, np.float32), np.asarray(bn_var, np.float32),
        np.asarray(w2, np.float32), np.asarray(b2, np.float32),
        np.asarray(w3, np.float32), np.asarray(b3, np.float32),
        np.asarray(w4, np.float32), np.asarray(b4, np.float32),
    )
    nc = _get_nc()
    in_maps = [
        {"x": np.ascontiguousarray(x[i * PER : (i + 1) * PER]), **wmap}
        for i in range(NCORES)
    ]
    res = run_bass_kernel_spmd(nc, in_maps, core_ids=list(range(NCORES)))
    return np.concatenate([res.results[i]["out"] for i in range(NCORES)], axis=0)


# revision 46
# speedup vs baseline: 1.0691x; 1.0313x over previous
"""CoordAttention kernel for Trainium2 (8 NeuronCores, data-parallel over batch).

Reference computation (per sample, inference):
  pools:  mean/max over W -> [C,H];  mean/max over H -> [C,W];  global mean/max -> [C]
  x_cat = concat(pools) -> [C, H+W+1, 2]
  y = BN(conv1x1(x_cat)) ; y = h_swish(y)
  a_h = sigmoid(conv(xh)), a_w = sigmoid(conv(xw)), a_c = sigmoid(conv(xc))
  out = x * a_w * a_h * a_c

v2 mapping onto one NeuronCore (2 samples each), fp16 datapath:
  - x loaded via casting SWDGE DMA (HBM fp32 -> SBUF fp16), chunked
    [128ch x 64h x 128w]
  - sum-pools: fp16 identity matmuls on PE (512-free, PSUM fp32 accum)
  - max-pools: fp16 tensor_tensor halving trees on DVE (2x fast mode)
  - conv1/BN folded host-side (fp32, tiny); sigmoids emit fp16
  - final multiply: ACT replicates a_h*a_c over w, DVE does two packed
    fp16 multiplies (2x); casting SWDGE DMA stores fp16 -> fp32 HBM
"""
import sys

for _p in ("/opt/trn_rl_repo", "/root/.axon_site/_ro/trn_rl_repo"):
    if _p not in sys.path:
        sys.path.insert(0, _p)

import copy as _copy
import numpy as np

import concourse.bass as bass
import concourse.mybir as mybir
import concourse.tile as tile

f32 = mybir.dt.float32
f16 = mybir.dt.float16
OP = mybir.AluOpType
AF = mybir.ActivationFunctionType
AX = mybir.AxisListType

N, C, H, W = 16, 256, 128, 128
TC = 8
NCORES = 8
PER = N // NCORES
EPS = 1e-5
CT = C // 128          # channel tiles per sample
NCH = 2                # h-chunks per channel tile
R = H // NCH           # rows per chunk
SS = H + W + 1         # pooled sequence length
JJ = 4                 # h-groups per sum-over-h matmul (free = JJ*W = 512)
MB = 8                 # w columns left after the PE sum-w stage


def _split_excess_waits(nc, limit=1):
    """This container's walrus accepts only one sync-wait per instruction;
    hoist extras onto same-engine drain carriers inserted just before."""
    m = nc.m
    newm = _copy.replace(m, functions=[])
    for fn in m.functions:
        newfn = _copy.replace(fn, blocks=[])
        newfn.set_allocations_from_list(fn.allocations)
        for blk in fn.blocks:
            out = []
            for inst in blk.instructions:
                si = inst.sync_info
                waits = list(si.on_wait) if si and si.on_wait else []
                if len(waits) > limit:
                    keep, excess = waits[-limit:], waits[: len(waits) - limit]
                    for gi, wchunk in enumerate(excess):
                        d = mybir.InstDrain(
                            name=f"{inst.name}-wsplit{gi}", ins=[], outs=[]
                        )
                        d.engine = inst.engine
                        d.sync_info = mybir.SyncInfo(on_wait=[wchunk], on_update=[])
                        out.append(d)
                    inst.sync_info = mybir.SyncInfo(
                        on_wait=keep, on_update=list(si.on_update or [])
                    )
                out.append(inst)
            newfn.blocks.append(_copy.replace(blk, instructions=out))
        newm.functions.append(newfn)
    nc.m = newm


def build_nc(per=PER, xp_bufs=8, split_waits=True):
    nc = bass.Bass()
    x_d = nc.declare_dram_parameter("x", [per, C, H, W], f32, isOutput=False)
    o_d = nc.declare_dram_parameter("out", [per, C, H, W], f32, isOutput=True)
    id_d = nc.declare_dram_parameter("ident16", [128, 128], f16, isOutput=False)
    w1_d = nc.declare_dram_parameter("w1t", [C, TC], f32, isOutput=False)
    b1_d = nc.declare_dram_parameter("b1f", [TC, 1], f32, isOutput=False)
    w2_d = nc.declare_dram_parameter("w2t", [TC, 2, C], f32, isOutput=False)
    w3_d = nc.declare_dram_parameter("w3t", [TC, 2, C], f32, isOutput=False)
    w4_d = nc.declare_dram_parameter("w4t", [TC, 2, C], f32, isOutput=False)
    b2_d = nc.declare_dram_parameter("b2r", [C, 1], f32, isOutput=False)
    b3_d = nc.declare_dram_parameter("b3r", [C, 1], f32, isOutput=False)
    b4_d = nc.declare_dram_parameter("b4r", [C, 1], f32, isOutput=False)
    cs_d = nc.declare_dram_parameter("colsc", [TC, SS], f32, isOutput=False)

    with tile.TileContext(nc) as tc:
        with (
            tc.tile_pool(name="const", bufs=1) as cp,
            tc.tile_pool(name="xp", bufs=xp_bufs) as xp,
            tc.tile_pool(name="scrp", bufs=3) as scr,
            tc.tile_pool(name="xcp", bufs=2) as xcp,
            tc.tile_pool(name="smp", bufs=2) as smp,
            tc.tile_pool(name="atp", bufs=3) as atp,
            tc.tile_pool(name="arp", bufs=2) as arp,
            tc.tile_pool(name="spool", bufs=2, space="PSUM") as spool,
            tc.tile_pool(name="pwpool", bufs=3, space="PSUM") as pwpool,
            tc.tile_pool(name="apsum", bufs=1, space="PSUM") as apsum,
        ):
            ident = cp.tile([128, 128], f16)
            nc.sync.dma_start(out=ident, in_=id_d[:, :])
            w1sb = cp.tile([128, CT, TC], f32)
            nc.sync.dma_start(
                out=w1sb, in_=w1_d.rearrange("(ct c) t -> c ct t", ct=CT)
            )
            b1sb = cp.tile([TC, 1], f32)
            nc.sync.dma_start(out=b1sb, in_=b1_d[:, :])
            colsc = cp.tile([TC, SS], f32)
            nc.sync.dma_start(out=colsc, in_=cs_d[:, :])
            wsb = {}
            for nm, d in (("w2", w2_d), ("w3", w3_d), ("w4", w4_d)):
                t = cp.tile([TC, 2, C], f32, tag=f"wsb_{nm}")
                nc.sync.dma_start(out=t, in_=d[:, :, :])
                wsb[nm] = t
            bsb = {}
            for nm, d in (("b2", b2_d), ("b3", b3_d), ("b4", b4_d)):
                t = cp.tile([128, CT, 1], f32, tag=f"bsb_{nm}")
                nc.sync.dma_start(out=t, in_=d.rearrange("(ct c) one -> c ct one", ct=CT))
                bsb[nm] = t

            # PE observes the identity once, so later matmuls carry one wait.
            warm = apsum.tile([128, 128], f32, tag="ap")
            nc.tensor.matmul(warm, ident, ident, start=True, stop=True)

            def load_chunk(s, ct, hh):
                # two half-loads per chunk: halves the first-tile latency
                # (the DMA engine pool runs ~2 transfers concurrently)
                t = xp.tile([128, R, W], f16, tag="x")
                for q in range(2):
                    h0 = hh * R + q * (R // 2)
                    nc.gpsimd.dma_start(
                        out=t[:, q * (R // 2) : (q + 1) * (R // 2), :],
                        in_=x_d[s, ct * 128 : (ct + 1) * 128, h0 : h0 + R // 2, :],
                    )
                return t

            xt = {}
            xc0, xc1 = {}, {}
            att = {}

            pstate = {}

            def emit_pools_pe(s, ct):
                """PE-only stream: sum-h accumulate + sum-w partial sums."""
                ch0, ch1 = xt[s, ct, 0], xt[s, ct, 1]
                xcat0 = xcp.tile([128, SS], f32, tag="xc0")
                xcat1 = xcp.tile([128, SS], f32, tag="xc1")
                psh = spool.tile([128, JJ, W], f32, tag="psh")
                # rhs reads JJ consecutive h-rows (1KB contiguous runs)
                for hh, ch in ((0, ch0), (1, ch1)):
                    chv = ch.rearrange("p (g jj) w -> p g jj w", jj=JJ)
                    gmax = R // JJ
                    for g in range(gmax):
                        nc.tensor.matmul(
                            psh,
                            ident,
                            chv[:, g],
                            start=(hh == 0 and g == 0),
                            stop=(hh == 1 and g == gmax - 1),
                        )
                pws = []
                for hh, ch in ((0, ch0), (1, ch1)):
                    pw = pwpool.tile([128, R, MB], f32, tag="pw")
                    for j in range(W // MB):
                        nc.tensor.matmul(
                            pw,
                            ident,
                            ch[:, :, j * MB : (j + 1) * MB],
                            start=(j == 0),
                            stop=(j == W // MB - 1),
                        )
                    pws.append(pw)
                pstate[s, ct] = (xcat0, xcat1, psh, pws)

            def emit_pools_trees(s, ct):
                """max-w trees for both chunks (no PE dependency)."""
                ch0, ch1 = xt[s, ct, 0], xt[s, ct, 1]
                xcat0, xcat1, psh, pws = pstate[s, ct]
                for hh, ch in ((0, ch0), (1, ch1)):
                    t = scr.tile([128, R * W // 2], f16, tag="scr")
                    t = t.rearrange("p (h w) -> p h w", h=R)
                    nc.vector.tensor_tensor(
                        out=t, in0=ch[:, :, 0 : W // 2], in1=ch[:, :, W // 2 : W],
                        op=OP.max,
                    )
                    g = W // 2
                    while g > 8:
                        nc.vector.tensor_tensor(
                            out=t[:, :, 0 : g // 2],
                            in0=t[:, :, 0 : g // 2],
                            in1=t[:, :, g // 2 : g],
                            op=OP.max,
                        )
                        g //= 2
                    nc.vector.reduce_max(
                        out=xcat1[:, hh * R : (hh + 1) * R], in_=t[:, :, 0:8],
                        axis=AX.X,
                    )

            def emit_pools_reduces(s, ct):
                """sum-w reduces straight from PSUM (needs PE stage done)."""
                xcat0, xcat1, psh, pws = pstate[s, ct]
                swv = xcat0[:, 0:H].rearrange("p (hh r) -> p hh r", hh=NCH)
                for hh in range(NCH):
                    nc.vector.reduce_sum(out=swv[:, hh], in_=pws[hh], axis=AX.X)

            def emit_pools_dve_a(s, ct):
                emit_pools_trees(s, ct)
                emit_pools_reduces(s, ct)

            def emit_pools_dve_b(s, ct):
                ch0, ch1 = xt[s, ct, 0], xt[s, ct, 1]
                xcat0, xcat1, psh, pws = pstate[s, ct]
                # max over h (DVE f16 tree): cross-chunk then halve
                mh = scr.tile([128, R * W // 2], f16, tag="scr")
                mh = mh.rearrange("p (h w) -> p h w", h=R // 2)
                nc.vector.tensor_tensor(
                    out=mh, in0=ch0[:, 0 : R // 2, :], in1=ch0[:, R // 2 : R, :],
                    op=OP.max,
                )
                nc.vector.tensor_tensor(
                    out=mh, in0=mh,
                    in1=ch1.rearrange("p (a b) w -> p a b w", a=2)[:, 0],
                    op=OP.max,
                )
                nc.vector.tensor_tensor(
                    out=mh, in0=mh,
                    in1=ch1.rearrange("p (a b) w -> p a b w", a=2)[:, 1],
                    op=OP.max,
                )
                g = R // 2
                while g > 1:
                    nc.vector.tensor_tensor(
                        out=mh[:, 0 : g // 2, :],
                        in0=mh[:, 0 : g // 2, :],
                        in1=mh[:, g // 2 : g, :],
                        op=OP.max,
                    )
                    g //= 2
                nc.vector.tensor_copy(out=xcat1[:, H : H + W], in_=mh[:, 0, :])
                # global max
                nc.vector.reduce_max(
                    out=xcat1[:, H + W : SS], in_=xcat1[:, 0:H], axis=AX.X
                )
                # fold sum-h partials (PSUM -> xcat0, raw sums)
                sh = xcat0[:, H : H + W]
                nc.vector.tensor_copy(out=sh, in_=psh[:, 0, :])
                nc.vector.tensor_add(out=sh, in0=sh, in1=psh[:, 1, :])
                nc.vector.tensor_add(out=sh, in0=sh, in1=psh[:, 2, :])
                nc.vector.tensor_add(out=sh, in0=sh, in1=psh[:, 3, :])
                # raw global sum
                nc.vector.reduce_sum(
                    out=xcat0[:, H + W : SS], in_=xcat0[:, 0:H], axis=AX.X
                )
                xc0[s, ct], xc1[s, ct] = xcat0, xcat1

            def emit_conv_att(s):
                # conv1 + h_swish (tiny, fp32); mean scaling applied as a
                # per-column scale after the matmul (k=0 path only)
                xh = smp.tile([TC, 2, SS], f32, tag="xh")
                for k in range(2):
                    yp = apsum.tile([TC, SS], f32, tag="y")
                    src = xc0 if k == 0 else xc1
                    for ct in range(CT):
                        nc.tensor.matmul(
                            yp,
                            w1sb[:, ct, :],
                            src[s, ct],
                            start=(ct == 0),
                            stop=(ct == CT - 1),
                        )
                    xhk = xh[:, k, :]
                    if k == 0:
                        nc.vector.tensor_mul(out=xhk, in0=yp, in1=colsc)
                        nc.vector.tensor_scalar_add(out=xhk, in0=xhk, scalar1=b1sb)
                    else:
                        nc.vector.tensor_scalar_add(out=xhk, in0=yp, scalar1=b1sb)
                    u = smp.tile([TC, SS], f32, tag="u")
                    nc.vector.tensor_scalar(
                        out=u, in0=xhk, scalar1=-3.0, scalar2=3.0,
                        op0=OP.max, op1=OP.min,
                    )
                    nc.vector.tensor_scalar_add(out=u, in0=u, scalar1=3.0)
                    nc.vector.tensor_mul(out=xhk, in0=xhk, in1=u)

                # attention maps (sigmoids in f16)
                for ct in range(CT):
                    ah = atp.tile([128, H], f16, tag="ah")
                    aw = atp.tile([128, W], f16, tag="aw")
                    ac = atp.tile([128, 1], f32, tag="ac")
                    for nm, bt, sl, dst in (
                        ("w2", "b2", slice(0, H), ah),
                        ("w3", "b3", slice(H, H + W), aw),
                        ("w4", "b4", slice(H + W, SS), ac),
                    ):
                        pp = apsum.tile([128, H], f32, tag="ap")
                        po = pp[:, 0 : dst.shape[-1]]
                        for k in range(2):
                            nc.tensor.matmul(
                                po,
                                wsb[nm][:, k, ct * 128 : (ct + 1) * 128],
                                xh[:, k, sl],
                                start=(k == 0),
                                stop=(k == 1),
                            )
                        nc.scalar.activation(
                            out=dst, in_=po, func=AF.Sigmoid,
                            bias=bsb[bt][:, ct, :], scale=1.0,
                        )
                    # ahc = a_h * a_c (f16, per-partition scalar)
                    nc.vector.tensor_scalar_mul(out=ah, in0=ah, scalar1=ac)
                    att[s, ct] = (ah, aw)
                    if ct == 0:
                        emit_repl(s, 0)

            reps = {}

            def emit_repl(s, ct):
                ah, _ = att[s, ct]
                for hh in range(NCH):
                    ahcrep = arp.tile([128, R, W], f16, tag="ahcrep")
                    nc.scalar.activation(
                        out=ahcrep,
                        in_=ah[:, hh * R : (hh + 1) * R]
                        .unsqueeze(2)
                        .to_broadcast([128, R, W]),
                        func=AF.Copy,
                    )
                    reps[s, ct, hh] = ahcrep

            def emit_apply(s, ct, hh, bcast_first=False):
                ah, aw = att[s, ct]
                HF = R // 2
                ch = xt[s, ct, hh]
                for half in range(2):
                    sl = slice(half * HF, (half + 1) * HF)
                    # x *= a_w (bcast mid), then *= ahc (packed) -- both 2x
                    nc.vector.tensor_tensor(
                        out=ch[:, sl],
                        in0=ch[:, sl],
                        in1=aw.unsqueeze(1).to_broadcast([128, HF, W]),
                        op=OP.mult,
                    )
                    if bcast_first and half == 0:
                        # stride-0 broadcast (1x) -- skips the repl dependency
                        # right after the attention chain
                        hc = ah[:, hh * R + half * HF : hh * R + (half + 1) * HF]
                        in1 = hc.unsqueeze(2).to_broadcast([128, HF, W])
                    else:
                        in1 = reps[s, ct, hh][:, sl]
                    nc.vector.tensor_tensor(
                        out=ch[:, sl], in0=ch[:, sl], in1=in1, op=OP.mult,
                    )
                    # casting store: f16 SBUF -> fp32 HBM
                    nc.gpsimd.dma_start(
                        out=o_d[
                            s,
                            ct * 128 : (ct + 1) * 128,
                            hh * R + half * HF : hh * R + (half + 1) * HF,
                            :,
                        ],
                        in_=ch[:, sl],
                    )

            # ---- software-pipelined schedule (per = 2) ----
            for s in range(per):
                for ct in range(CT):
                    for hh in range(NCH):
                        xt[s, ct, hh] = load_chunk(s, ct, hh)
            emit_pools_pe(0, 0)
            emit_pools_dve_a(0, 0)
            emit_pools_dve_b(0, 0)
            emit_pools_pe(0, 1)
            emit_pools_trees(0, 1)
            emit_pools_reduces(0, 1)
            emit_pools_dve_b(0, 1)
            emit_pools_pe(1, 0)       # PE: before conv1 (conv1 waits on DVE)
            emit_pools_trees(1, 0)    # DVE: fills the conv(0)-chain latency
            emit_conv_att(0)          # includes repl(0, 0)
            emit_pools_pe(1, 1)
            emit_apply(0, 0, 0, bcast_first=True)
            emit_apply(0, 0, 1)
            emit_repl(0, 1)
            emit_apply(0, 1, 0)
            emit_apply(0, 1, 1)
            emit_pools_reduces(1, 0)
            emit_pools_dve_b(1, 0)
            emit_pools_trees(1, 1)
            emit_pools_reduces(1, 1)
            emit_pools_dve_b(1, 1)
            emit_conv_att(1)          # includes repl(1, 0)
            emit_apply(1, 0, 0, bcast_first=True)
            emit_apply(1, 0, 1)
            emit_repl(1, 1)
            emit_apply(1, 1, 0)
            emit_apply(1, 1, 1)

    if split_waits:
        _split_excess_waits(nc)
    return nc


def prep_weights(w1, b1, bn_gamma, bn_beta, bn_mean, bn_var, w2, b2, w3, b3, w4, b4):
    inv = (bn_gamma / np.sqrt(bn_var + EPS)).astype(np.float32)
    w1f = (w1 * inv[:, None]).astype(np.float32)          # [TC, C]
    b1f = ((b1 - bn_mean) * inv + bn_beta).astype(np.float32)
    def pack(wk):  # [C, TC, 2] -> [TC, 2, C], with the h_swish /6 folded in
        return np.ascontiguousarray(wk.transpose(1, 2, 0) / 6.0).astype(np.float32)
    colvec = np.concatenate([
        np.full(H, 1.0 / W), np.full(W, 1.0 / H), np.full(1, 1.0 / (H * W))
    ]).astype(np.float32)
    return dict(
        ident16=np.eye(128, dtype=np.float16),
        w1t=np.ascontiguousarray(w1f.T),                  # [C, TC]
        b1f=b1f.reshape(TC, 1),
        colsc=np.ascontiguousarray(np.tile(colvec, (TC, 1))),
        w2t=pack(w2), w3t=pack(w3), w4t=pack(w4),
        b2r=b2.reshape(C, 1).astype(np.float32),
        b3r=b3.reshape(C, 1).astype(np.float32),
        b4r=b4.reshape(C, 1).astype(np.float32),
    )


_NC_CACHE = {}


def _get_nc():
    if "nc" not in _NC_CACHE:
        _NC_CACHE["nc"] = build_nc()
    return _NC_CACHE["nc"]


def kernel(x, w1, b1, bn_gamma, bn_beta, bn_mean, bn_var, w2, b2, w3, b3, w4, b4):
    from concourse.bass_utils import run_bass_kernel_spmd

    x = np.asarray(x, dtype=np.float32)
    wmap = prep_weights(
        np.asarray(w1, np.float32), np.asarray(b1, np.float32),
        np.asarray(bn_gamma, np.float32), np.asarray(bn_beta, np.float32),
        np.asarray(bn_mean, np.float32), np.asarray(bn_var, np.float32),
        np.asarray(w2, np.float32), np.asarray(b2, np.float32),
        np.asarray(w3, np.float32), np.asarray(b3, np.float32),
        np.asarray(w4, np.float32), np.asarray(b4, np.float32),
    )
    nc = _get_nc()
    in_maps = [
        {"x": np.ascontiguousarray(x[i * PER : (i + 1) * PER]), **wmap}
        for i in range(NCORES)
    ]
    res = run_bass_kernel_spmd(nc, in_maps, core_ids=list(range(NCORES)))
    return np.concatenate([res.results[i]["out"] for i in range(NCORES)], axis=0)
